# revision 22
# baseline (speedup 1.0000x reference)
"""Fused LayerNorm + MHA + projections on 8 TRN2 NeuronCores.

Problem (hardcoded): x [4, 2048, 1024] f32, 16 heads x 64 dh, inner 1024.
Sharding: core c = (batch b = c//2, head-group g = c%2, 8 heads each).
Each core returns a partial output [2048, 1024] (its heads' contribution
to the out-projection); host sums the pair per batch and adds b_out.

Per-core math:
  LN folded into QKV:  qkv = rstd*(x @ (g*W)) + (-mu*rstd)*u + r
  (u = sum_f g*W, r = sum_f b*W are host-precomputed aug rows; the rstd
  row is applied at PSUM eviction).
  Attention in the "transposed world": scores^T [keys, q] strips, exp on
  ScalarE (one op per [128, 2048] strip), attn@v with lhsT = [v_h | ones]
  (65 cols) so softmax denominators accumulate in PSUM row 64 for free.
"""

import numpy as np
import ml_dtypes

B, N, D = 4, 2048, 1024
HEADS_TOT, DH = 16, 64
HL = 8               # local heads per core
IN_L = HL * DH       # 512 local inner dim
NCORES = 8
P = 128
KT = N // P          # 16 key tiles
NCH = N // 512       # 4 q chunks of 512
EPS = 1e-5
SCALE = DH ** -0.5

BF16 = ml_dtypes.bfloat16

_compiled = None  # (nc, names) cache


def _build():
    import contextlib
    import concourse.mybir as mybir
    import concourse.tile as tile
    from concourse import bacc

    fp32 = mybir.dt.float32
    bf16 = mybir.dt.bfloat16
    AF = mybir.ActivationFunctionType

    nc = bacc.Bacc("TRN2", target_bir_lowering=False, debug=False)

    with tile.TileContext(nc) as tc, contextlib.ExitStack() as ctx:
        dram = ctx.enter_context(tc.tile_pool(name="dram", bufs=1, space="DRAM"))
        xt_d = dram.tile([D, N], bf16, kind="ExternalInput")        # x[b].T
        wqk_d = dram.tile([D + 2, 2 * IN_L], bf16, kind="ExternalInput")
        wv_d = dram.tile([D + 2, IN_L], bf16, kind="ExternalInput")
        wo_d = dram.tile([IN_L, D], bf16, kind="ExternalInput")
        out_d = dram.tile([N, D], fp32, kind="ExternalOutput")
        ones_d = dram.tile([1, N], bf16, kind="ExternalInput")
        scr_d = dram.tile([1, N], fp32)  # bounce buffer for rstd restripe

        const = ctx.enter_context(tc.tile_pool(name="const", bufs=1))
        # ---- static SBUF tensors ----
        xt = [const.tile([P, N], bf16, tag=f"xt{i}", name=f"xt{i}") for i in range(8)]
        wqk = [const.tile([P, 2 * IN_L], bf16, tag=f"wqk{i}", name=f"wqk{i}") for i in range(8)]
        wqk_aug = const.tile([2, 2 * IN_L], bf16, tag="wqk_aug", name="wqk_aug")
        wv = [const.tile([P, IN_L], bf16, tag=f"wv{i}", name=f"wv{i}") for i in range(8)]
        wv_aug = const.tile([2, IN_L], bf16, tag="wv_aug", name="wv_aug")
        wo = [const.tile([P, D], bf16, tag=f"wo{i}", name=f"wo{i}") for i in range(4)]
        ones_col = const.tile([P, 1], bf16, tag="ones_col", name="ones_col")
        aug = const.tile([2, N], bf16, tag="aug", name="aug")  # [nmr; ones]
        # qkT tiles: 0-3 = q pairs [q_2m|q_2m+1], 4-7 = k pairs
        qkT = [const.tile([P, N], bf16, tag=f"qkT{i}", name=f"qkT{i}") for i in range(8)]
        # v with ones col: head h at cols h*65..h*65+64 (col 64 = ones)
        vt = [const.tile([P, HL * (DH + 1)], bf16, tag=f"vt{i}", name=f"vt{i}")
              for i in range(KT)]
        attT = [const.tile([P, N], bf16, tag=f"attT{i}", name=f"attT{i}") for i in range(4)]
        # [1, N] f32 stat rows; all at base partition 0 (engine ops require
        # matching start partitions across operands)
        rowA = const.tile([1, N], fp32, tag="rowA", name="rowA")  # mu -> sums
        rowB = const.tile([1, N], fp32, tag="rowB", name="rowB")  # sumsq->rstd
        rowC = const.tile([1, N], fp32, tag="rowC", name="rowC")  # scratch
        rstd_b = const.tile([P, N], fp32, tag="rstd_b", name="rstd_b")
        rstd_col = const.tile([P, KT], fp32, tag="rstd_col", name="rstd_col")

        # ---- DMA inputs ----
        for i in range(8):
            nc.sync.dma_start(xt[i][:], xt_d[i * P:(i + 1) * P, :])
        for i in range(8):
            nc.sync.dma_start(wqk[i][:], wqk_d[i * P:(i + 1) * P, :])
        nc.sync.dma_start(wqk_aug[:], wqk_d[D:D + 2, :])
        for i in range(8):
            nc.sync.dma_start(wv[i][:], wv_d[i * P:(i + 1) * P, :])
        nc.sync.dma_start(wv_aug[:], wv_d[D:D + 2, :])
        for i in range(4):
            nc.sync.dma_start(wo[i][:], wo_d[i * P:(i + 1) * P, :])

        nc.vector.memset(ones_col[:], 1.0)
        nc.sync.dma_start(aug[1:2, :], ones_d[:])

        # ---- Phase 1: LN stats ----
        with tc.tile_pool(name="sq", bufs=2) as sq_pool, \
             tc.tile_pool(name="st_psum", bufs=1, space="PSUM") as st_psum:
            ps1 = [st_psum.tile([1, 512], fp32, tag=f"s1{c}", name=f"s1{c}")
                   for c in range(NCH)]
            ps2 = [st_psum.tile([1, 512], fp32, tag=f"s2{c}", name=f"s2{c}")
                   for c in range(NCH)]
            for i in range(8):
                s = sq_pool.tile([P, N], bf16, tag="xsq", name="xsq")
                nc.vector.tensor_mul(s[:], xt[i][:], xt[i][:])
                for c in range(NCH):
                    cs = slice(c * 512, (c + 1) * 512)
                    nc.tensor.matmul(ps1[c][:], ones_col[:], xt[i][:, cs],
                                     start=(i == 0), stop=(i == 7))
                    nc.tensor.matmul(ps2[c][:], ones_col[:], s[:, cs],
                                     start=(i == 0), stop=(i == 7))
            for c in range(NCH):
                cs = slice(c * 512, (c + 1) * 512)
                # fused evict+scale: mu = sum_x/D, ex2 = sum_x2/D
                nc.vector.tensor_scalar_mul(rowA[0:1, cs], ps1[c][:], 1.0 / D)
                nc.vector.tensor_scalar_mul(rowB[0:1, cs], ps2[c][:], 1.0 / D)
        # row math (all [1, N] at base partition 0; different tensors)
        nc.vector.tensor_mul(rowC[:], rowA[:], rowA[:])        # mu^2
        nc.vector.tensor_sub(rowB[:], rowB[:], rowC[:])        # var
        # rstd = exp(-0.5 * ln(var + eps))
        eps_t = const.tile([1, 1], fp32, tag="eps_t", name="eps_t")
        nc.vector.memset(eps_t[:], EPS)
        nc.scalar.activation(rowB[:], rowB[:], AF.Ln, bias=eps_t[:])
        nc.scalar.activation(rowB[:], rowB[:], AF.Exp, scale=-0.5)  # rstd
        # aug row 0 = -mu * rstd (bf16)
        nc.vector.tensor_mul(rowC[:], rowA[:], rowB[:])
        nc.vector.tensor_scalar_mul(aug[0:1, :], rowC[:], -1.0)
        # rstd broadcast down partitions + column-form restripe (DRAM bounce)
        nc.gpsimd.partition_broadcast(rstd_b[:], rowB[:])
        nc.sync.dma_start(scr_d[:], rowB[:])
        nc.sync.dma_start(rstd_col[:],
                          scr_d[:].rearrange("o (t p) -> (o p) t", p=P))

        # ---- Phase 2+3: QKV interleaved with per-head attention ----
        # Emission order: qk pair 0 -> v -> heads 0,1 -> qk pair 1 -> heads
        # 2,3 -> ...  so ScalarE (exp, the binding engine) starts ~4x earlier
        # and PE fills exp-wait gaps with the next pair's QKV matmuls.
        NH = N // 2  # q processed in halves so scores PSUM double-buffers
        with tc.tile_pool(name="qkv_psum", bufs=2, space="PSUM") as qkv_psum, \
             tc.tile_pool(name="s_psum", bufs=2, space="PSUM") as s_psum, \
             tc.tile_pool(name="av_psum", bufs=1, space="PSUM") as av_psum, \
             tc.tile_pool(name="expp", bufs=4) as exp_pool, \
             tc.tile_pool(name="norm", bufs=1) as norm_pool:

            def emit_qk_pair(m):
                for j in (m, 4 + m):
                    js = slice(j * P, (j + 1) * P)
                    for c in range(NCH):
                        cs = slice(c * 512, (c + 1) * 512)
                        ps = qkv_psum.tile([P, 512], fp32, tag="qkv",
                                           name="qkv")
                        for kk in range(8):
                            nc.tensor.matmul(ps[:], wqk[kk][:, js],
                                             xt[kk][:, cs],
                                             start=(kk == 0), stop=False)
                        nc.tensor.matmul(ps[:], wqk_aug[:, js], aug[:, cs],
                                         start=False, stop=True)
                        nc.vector.tensor_mul(qkT[j][:, cs], ps[:],
                                             rstd_b[:, cs])

            def emit_v():
                for tt in range(KT):
                    ts_ = slice(tt * P, (tt + 1) * P)
                    ps = qkv_psum.tile([P, IN_L], fp32, tag="qkv", name="qkv")
                    for kk in range(8):
                        nc.tensor.matmul(ps[:], xt[kk][:, ts_], wv[kk][:],
                                         start=(kk == 0), stop=False)
                    nc.tensor.matmul(ps[:], aug[:, ts_], wv_aug[:],
                                     start=False, stop=True)
                    # evict + rstd scale into strided [128,8,65] view
                    vview = vt[tt][:].rearrange("p (h e) -> p h e", e=DH + 1)
                    nc.vector.tensor_scalar_mul(
                        vview[:, :, 0:DH],
                        ps[:].rearrange("p (h e) -> p h e", e=DH),
                        rstd_col[:, tt:tt + 1])
                    nc.vector.tensor_copy(
                        vview[:, :, DH:DH + 1],
                        ones_col[:, 0:1].rearrange("p (a o) -> p a o", a=1)
                        .broadcast_to((P, HL, 1)))

            def emit_head(h):
                m, half = h // 2, h % 2
                qtile, ktile = qkT[m], qkT[4 + m]
                hp = slice(half * DH, (half + 1) * DH)
                for qh in range(2):
                    qs0 = qh * NH
                    av = av_psum.tile([DH + 1, NH], fp32, tag="av", name="av")
                    for kt in range(KT):
                        ks = slice(kt * P, (kt + 1) * P)
                        sp = s_psum.tile([P, NH], fp32, tag="s", name="s")
                        for c in range(2):
                            nc.tensor.matmul(
                                sp[:, c * 512:(c + 1) * 512], ktile[hp, ks],
                                qtile[hp, qs0 + c * 512:qs0 + (c + 1) * 512],
                                start=True, stop=True)
                        et = exp_pool.tile([P, NH], bf16, tag="exp",
                                           name="exp")
                        nc.scalar.activation(et[:], sp[:], AF.Exp)
                        vl = vt[kt][:, h * (DH + 1):(h + 1) * (DH + 1)]
                        for c in range(2):
                            nc.tensor.matmul(
                                av[:, c * 512:(c + 1) * 512], vl,
                                et[:, c * 512:(c + 1) * 512],
                                start=(kt == 0), stop=(kt == KT - 1))
                    # copy whole av to SBUF immediately so the PSUM bank is
                    # released fast (av pool runs with a single buffer), then
                    # normalize from the SBUF copy: recip of sums row (p64 ->
                    # DMA to p0 -> bcast to 64 partitions) times rows 0..63
                    unorm = norm_pool.tile([DH + 1, NH], fp32, tag="unorm",
                                           name="unorm", bufs=2)
                    recip = norm_pool.tile([DH, NH], fp32, tag="recip",
                                           name="recip")
                    natt = norm_pool.tile([DH, NH], bf16, tag="natt",
                                          name="natt")
                    rh = slice(qs0, qs0 + NH)
                    nc.vector.tensor_copy(unorm[:], av[:])
                    nc.sync.dma_start(rowA[0:1, rh], unorm[DH:DH + 1, :])
                    nc.vector.reciprocal_approx_fast(rowA[0:1, rh],
                                                     rowA[0:1, rh])
                    nc.gpsimd.partition_broadcast(recip[:], rowA[0:1, rh])
                    nc.vector.tensor_mul(natt[:], unorm[0:DH, :], recip[:])
                    nc.sync.dma_start(attT[m][hp, rh], natt[:])

            emit_qk_pair(0)
            emit_v()
            for m in range(4):
                if m > 0:
                    emit_qk_pair(m)
                # heads jump the priority queue past the v/qk filler blocks:
                # their scores gate ScalarE (the binding engine); v and the
                # next qk pair fill PE gaps instead of blocking exp
                with tc.high_priority(offset=280):
                    emit_head(2 * m)
                with tc.high_priority(offset=280):
                    emit_head(2 * m + 1)

        # ---- Phase 4: out projection ----
        with tc.tile_pool(name="pr_psum", bufs=2, space="PSUM") as pr_psum, \
             tc.tile_pool(name="ostage", bufs=3) as ostage:
            for tt in range(KT):
                ts_ = slice(tt * P, (tt + 1) * P)
                ot = ostage.tile([P, D], fp32, tag="ot", name="ot")
                for c in range(2):
                    cs = slice(c * 512, (c + 1) * 512)
                    ps = pr_psum.tile([P, 512], fp32, tag="pr", name="pr")
                    for m in range(4):
                        nc.tensor.matmul(ps[:], attT[m][:, ts_], wo[m][:, cs],
                                         start=(m == 0), stop=(m == 3))
                    nc.vector.tensor_copy(ot[:, cs], ps[:])
                nc.sync.dma_start(out_d[ts_, :], ot[:])

    nc.compile()
    names = dict(xt=xt_d.name, wqk=wqk_d.name, wv=wv_d.name, wo=wo_d.name,
                 ones=ones_d.name, out=out_d.name)
    return nc, names


def _prep_in_maps(x, ln_g, ln_b, w_qkv, w_out, b_out, names):
    x = np.asarray(x, dtype=np.float32)
    ln_g = np.asarray(ln_g, dtype=np.float64)
    ln_b = np.asarray(ln_b, dtype=np.float64)
    w_qkv = np.asarray(w_qkv, dtype=np.float64)
    w_out = np.asarray(w_out, dtype=np.float32)
    b_out = np.asarray(b_out, dtype=np.float32)

    wq = w_qkv[:, 0 * D:1 * D] * SCALE   # fold softmax scale into q
    wk = w_qkv[:, 1 * D:2 * D]
    wv_ = w_qkv[:, 2 * D:3 * D]

    in_maps = []
    for c in range(NCORES):
        b, g = c // 2, c % 2
        heads = range(g * HL, (g + 1) * HL)
        # wqk cols: [q_h0 q_h1 | ... | q_h6 q_h7 | k_h0 k_h1 | ...] pair tiles
        qcols = np.concatenate([wq[:, h * DH:(h + 1) * DH] for h in heads], axis=1)
        kcols = np.concatenate([wk[:, h * DH:(h + 1) * DH] for h in heads], axis=1)
        vcols = np.concatenate([wv_[:, h * DH:(h + 1) * DH] for h in heads], axis=1)
        wqk_c = np.concatenate([qcols, kcols], axis=1)          # [D, 1024]
        wqk_aug = np.concatenate([
            ln_g[:, None] * wqk_c,
            (ln_g @ wqk_c)[None, :],
            (ln_b @ wqk_c)[None, :]], axis=0)                   # [D+2, 1024]
        wv_aug = np.concatenate([
            ln_g[:, None] * vcols,
            (ln_g @ vcols)[None, :],
            (ln_b @ vcols)[None, :]], axis=0)                   # [D+2, 512]
        wo_c = w_out[g * IN_L:(g + 1) * IN_L, :]                # [512, D]
        in_maps.append({
            names["xt"]: np.ascontiguousarray(x[b].T).astype(BF16),
            names["wqk"]: wqk_aug.astype(np.float32).astype(BF16),
            names["wv"]: wv_aug.astype(np.float32).astype(BF16),
            names["wo"]: np.ascontiguousarray(wo_c).astype(BF16),
            names["ones"]: np.ones((1, N), dtype=BF16),
        })
    return in_maps


def _gather(res, names, b_out):
    out = np.empty((B, N, D), dtype=np.float32)
    for b in range(B):
        out[b] = res.results[2 * b][names["out"]] \
            + res.results[2 * b + 1][names["out"]] + np.asarray(b_out, np.float32)
    return out


def kernel(x, ln_g, ln_b, w_qkv, w_out, b_out):
    global _compiled
    from concourse.bass_utils import run_bass_kernel_spmd

    if _compiled is None:
        _compiled = _build()
    nc, names = _compiled
    in_maps = _prep_in_maps(x, ln_g, ln_b, w_qkv, w_out, b_out, names)
    res = run_bass_kernel_spmd(nc, in_maps, core_ids=list(range(NCORES)))
    return _gather(res, names, b_out)


def run_traced(x, ln_g, ln_b, w_qkv, w_out, b_out):
    """Test helper: run with NTFF tracing enabled, return BassKernelResults."""
    global _compiled
    from concourse.bass_utils import run_bass_kernel_spmd

    if _compiled is None:
        _compiled = _build()
    nc, names = _compiled
    in_maps = _prep_in_maps(x, ln_g, ln_b, w_qkv, w_out, b_out, names)
    return run_bass_kernel_spmd(nc, in_maps, core_ids=list(range(NCORES)),
                                trace=True)


# revision 23
# speedup vs baseline: 1.0037x; 1.0037x over previous
"""Fused LayerNorm + MHA + projections on 8 TRN2 NeuronCores.

Problem (hardcoded): x [4, 2048, 1024] f32, 16 heads x 64 dh, inner 1024.
Sharding: core c = (batch b = c//2, head-group g = c%2, 8 heads each).
Each core returns a partial output [2048, 1024] (its heads' contribution
to the out-projection); host sums the pair per batch and adds b_out.

Per-core math:
  LN folded into QKV:  qkv = rstd*(x @ (g*W)) + (-mu*rstd)*u + r
  (u = sum_f g*W, r = sum_f b*W are host-precomputed aug rows; the rstd
  row is applied at PSUM eviction).
  Attention in the "transposed world": scores^T [keys, q] strips, exp on
  ScalarE (one op per [128, 2048] strip), attn@v with lhsT = [v_h | ones]
  (65 cols) so softmax denominators accumulate in PSUM row 64 for free.
"""

import numpy as np
import ml_dtypes

B, N, D = 4, 2048, 1024
HEADS_TOT, DH = 16, 64
HL = 8               # local heads per core
IN_L = HL * DH       # 512 local inner dim
NCORES = 8
P = 128
KT = N // P          # 16 key tiles
NCH = N // 512       # 4 q chunks of 512
EPS = 1e-5
SCALE = DH ** -0.5

BF16 = ml_dtypes.bfloat16

_compiled = None  # (nc, names) cache


def _build():
    import contextlib
    import concourse.mybir as mybir
    import concourse.tile as tile
    from concourse import bacc

    fp32 = mybir.dt.float32
    bf16 = mybir.dt.bfloat16
    AF = mybir.ActivationFunctionType

    nc = bacc.Bacc("TRN2", target_bir_lowering=False, debug=False)

    with tile.TileContext(nc) as tc, contextlib.ExitStack() as ctx:
        dram = ctx.enter_context(tc.tile_pool(name="dram", bufs=1, space="DRAM"))
        xt_d = dram.tile([D, N], bf16, kind="ExternalInput")        # x[b].T
        wqk_d = dram.tile([D + 2, 2 * IN_L], bf16, kind="ExternalInput")
        wv_d = dram.tile([D + 2, IN_L], bf16, kind="ExternalInput")
        wo_d = dram.tile([IN_L, D], bf16, kind="ExternalInput")
        out_d = dram.tile([N, D], fp32, kind="ExternalOutput")
        ones_d = dram.tile([1, N], bf16, kind="ExternalInput")
        scr_d = dram.tile([1, N], fp32)  # bounce buffer for rstd restripe

        const = ctx.enter_context(tc.tile_pool(name="const", bufs=1))
        # ---- static SBUF tensors ----
        xt = [const.tile([P, N], bf16, tag=f"xt{i}", name=f"xt{i}") for i in range(8)]
        wqk = [const.tile([P, 2 * IN_L], bf16, tag=f"wqk{i}", name=f"wqk{i}") for i in range(8)]
        wqk_aug = const.tile([2, 2 * IN_L], bf16, tag="wqk_aug", name="wqk_aug")
        wv = [const.tile([P, IN_L], bf16, tag=f"wv{i}", name=f"wv{i}") for i in range(8)]
        wv_aug = const.tile([2, IN_L], bf16, tag="wv_aug", name="wv_aug")
        wo = [const.tile([P, D], bf16, tag=f"wo{i}", name=f"wo{i}") for i in range(4)]
        ones_col = const.tile([P, 1], bf16, tag="ones_col", name="ones_col")
        aug = const.tile([2, N], bf16, tag="aug", name="aug")  # [nmr; ones]
        # qkT tiles: 0-3 = q pairs [q_2m|q_2m+1], 4-7 = k pairs
        qkT = [const.tile([P, N], bf16, tag=f"qkT{i}", name=f"qkT{i}") for i in range(8)]
        # v with ones col: head h at cols h*65..h*65+64 (col 64 = ones)
        vt = [const.tile([P, HL * (DH + 1)], bf16, tag=f"vt{i}", name=f"vt{i}")
              for i in range(KT)]
        attT = [const.tile([P, N], bf16, tag=f"attT{i}", name=f"attT{i}") for i in range(4)]
        # [1, N] f32 stat rows; all at base partition 0 (engine ops require
        # matching start partitions across operands)
        rowA = const.tile([1, N], fp32, tag="rowA", name="rowA")  # mu -> sums
        rowB = const.tile([1, N], fp32, tag="rowB", name="rowB")  # sumsq->rstd
        rowC = const.tile([1, N], fp32, tag="rowC", name="rowC")  # scratch
        rstd_b = const.tile([P, N], fp32, tag="rstd_b", name="rstd_b")
        rstd_col = const.tile([P, KT], fp32, tag="rstd_col", name="rstd_col")

        # ---- DMA inputs ----
        for i in range(8):
            nc.sync.dma_start(xt[i][:], xt_d[i * P:(i + 1) * P, :])
        for i in range(8):
            nc.sync.dma_start(wqk[i][:], wqk_d[i * P:(i + 1) * P, :])
        nc.sync.dma_start(wqk_aug[:], wqk_d[D:D + 2, :])
        for i in range(8):
            nc.sync.dma_start(wv[i][:], wv_d[i * P:(i + 1) * P, :])
        nc.sync.dma_start(wv_aug[:], wv_d[D:D + 2, :])
        for i in range(4):
            nc.sync.dma_start(wo[i][:], wo_d[i * P:(i + 1) * P, :])

        nc.vector.memset(ones_col[:], 1.0)
        nc.sync.dma_start(aug[1:2, :], ones_d[:])

        # ---- Phase 1: LN stats ----
        with tc.tile_pool(name="sq", bufs=2) as sq_pool, \
             tc.tile_pool(name="st_psum", bufs=1, space="PSUM") as st_psum:
            ps1 = [st_psum.tile([1, 512], fp32, tag=f"s1{c}", name=f"s1{c}")
                   for c in range(NCH)]
            ps2 = [st_psum.tile([1, 512], fp32, tag=f"s2{c}", name=f"s2{c}")
                   for c in range(NCH)]
            for i in range(8):
                s = sq_pool.tile([P, N], bf16, tag="xsq", name="xsq")
                nc.vector.tensor_mul(s[:], xt[i][:], xt[i][:])
                for c in range(NCH):
                    cs = slice(c * 512, (c + 1) * 512)
                    nc.tensor.matmul(ps1[c][:], ones_col[:], xt[i][:, cs],
                                     start=(i == 0), stop=(i == 7))
                    nc.tensor.matmul(ps2[c][:], ones_col[:], s[:, cs],
                                     start=(i == 0), stop=(i == 7))
            for c in range(NCH):
                cs = slice(c * 512, (c + 1) * 512)
                # fused evict+scale: mu = sum_x/D, ex2 = sum_x2/D
                nc.vector.tensor_scalar_mul(rowA[0:1, cs], ps1[c][:], 1.0 / D)
                nc.vector.tensor_scalar_mul(rowB[0:1, cs], ps2[c][:], 1.0 / D)
        # row math (all [1, N] at base partition 0; different tensors)
        nc.vector.tensor_mul(rowC[:], rowA[:], rowA[:])        # mu^2
        nc.vector.tensor_sub(rowB[:], rowB[:], rowC[:])        # var
        # rstd = exp(-0.5 * ln(var + eps))
        eps_t = const.tile([1, 1], fp32, tag="eps_t", name="eps_t")
        nc.vector.memset(eps_t[:], EPS)
        nc.scalar.activation(rowB[:], rowB[:], AF.Ln, bias=eps_t[:])
        nc.scalar.activation(rowB[:], rowB[:], AF.Exp, scale=-0.5)  # rstd
        # aug row 0 = -mu * rstd (bf16)
        nc.vector.tensor_mul(rowC[:], rowA[:], rowB[:])
        nc.vector.tensor_scalar_mul(aug[0:1, :], rowC[:], -1.0)
        # rstd broadcast down partitions + column-form restripe (DRAM bounce)
        nc.gpsimd.partition_broadcast(rstd_b[:], rowB[:])
        nc.sync.dma_start(scr_d[:], rowB[:])
        nc.sync.dma_start(rstd_col[:],
                          scr_d[:].rearrange("o (t p) -> (o p) t", p=P))

        # ---- Phase 2+3: QKV interleaved with per-head attention ----
        # Emission order: qk pair 0 -> v -> heads 0,1 -> qk pair 1 -> heads
        # 2,3 -> ...  so ScalarE (exp, the binding engine) starts ~4x earlier
        # and PE fills exp-wait gaps with the next pair's QKV matmuls.
        NH = N // 2  # q processed in halves so scores PSUM double-buffers
        with tc.tile_pool(name="qkv_psum", bufs=2, space="PSUM") as qkv_psum, \
             tc.tile_pool(name="s_psum", bufs=2, space="PSUM") as s_psum, \
             tc.tile_pool(name="av_psum", bufs=1, space="PSUM") as av_psum, \
             tc.tile_pool(name="expp", bufs=4) as exp_pool, \
             tc.tile_pool(name="norm", bufs=1) as norm_pool:

            def emit_qk_pair(m):
                for j in (m, 4 + m):
                    js = slice(j * P, (j + 1) * P)
                    for c in range(NCH):
                        cs = slice(c * 512, (c + 1) * 512)
                        ps = qkv_psum.tile([P, 512], fp32, tag="qkv",
                                           name="qkv")
                        for kk in range(8):
                            nc.tensor.matmul(ps[:], wqk[kk][:, js],
                                             xt[kk][:, cs],
                                             start=(kk == 0), stop=False)
                        nc.tensor.matmul(ps[:], wqk_aug[:, js], aug[:, cs],
                                         start=False, stop=True)
                        nc.vector.tensor_mul(qkT[j][:, cs], ps[:],
                                             rstd_b[:, cs])

            def emit_v():
                for tt in range(KT):
                    ts_ = slice(tt * P, (tt + 1) * P)
                    ps = qkv_psum.tile([P, IN_L], fp32, tag="qkv", name="qkv")
                    for kk in range(8):
                        nc.tensor.matmul(ps[:], xt[kk][:, ts_], wv[kk][:],
                                         start=(kk == 0), stop=False)
                    nc.tensor.matmul(ps[:], aug[:, ts_], wv_aug[:],
                                     start=False, stop=True)
                    # evict + rstd scale into strided [128,8,65] view
                    vview = vt[tt][:].rearrange("p (h e) -> p h e", e=DH + 1)
                    nc.vector.tensor_scalar_mul(
                        vview[:, :, 0:DH],
                        ps[:].rearrange("p (h e) -> p h e", e=DH),
                        rstd_col[:, tt:tt + 1])
                    nc.vector.tensor_copy(
                        vview[:, :, DH:DH + 1],
                        ones_col[:, 0:1].rearrange("p (a o) -> p a o", a=1)
                        .broadcast_to((P, HL, 1)))

            def emit_head(h):
                m, half = h // 2, h % 2
                qtile, ktile = qkT[m], qkT[4 + m]
                hp = slice(half * DH, (half + 1) * DH)
                for qh in range(2):
                    qs0 = qh * NH
                    av = av_psum.tile([DH + 1, NH], fp32, tag="av", name="av")
                    for kt in range(KT):
                        ks = slice(kt * P, (kt + 1) * P)
                        sp = s_psum.tile([P, NH], fp32, tag="s", name="s")
                        for c in range(2):
                            nc.tensor.matmul(
                                sp[:, c * 512:(c + 1) * 512], ktile[hp, ks],
                                qtile[hp, qs0 + c * 512:qs0 + (c + 1) * 512],
                                start=True, stop=True)
                        et = exp_pool.tile([P, NH], bf16, tag="exp",
                                           name="exp")
                        nc.scalar.activation(et[:], sp[:], AF.Exp)
                        vl = vt[kt][:, h * (DH + 1):(h + 1) * (DH + 1)]
                        for c in range(2):
                            nc.tensor.matmul(
                                av[:, c * 512:(c + 1) * 512], vl,
                                et[:, c * 512:(c + 1) * 512],
                                start=(kt == 0), stop=(kt == KT - 1))
                    # copy whole av to SBUF immediately so the PSUM bank is
                    # released fast (av pool runs with a single buffer), then
                    # normalize from the SBUF copy: recip of sums row (p64 ->
                    # DMA to p0 -> bcast to 64 partitions) times rows 0..63
                    unorm = norm_pool.tile([DH + 1, NH], fp32, tag="unorm",
                                           name="unorm", bufs=2)
                    recip = norm_pool.tile([DH, NH], fp32, tag="recip",
                                           name="recip")
                    natt = norm_pool.tile([DH, NH], bf16, tag="natt",
                                          name="natt")
                    rh = slice(qs0, qs0 + NH)
                    nc.vector.tensor_copy(unorm[:], av[:])
                    nc.sync.dma_start(rowA[0:1, rh], unorm[DH:DH + 1, :])
                    nc.vector.reciprocal_approx_fast(rowA[0:1, rh],
                                                     rowA[0:1, rh])
                    nc.gpsimd.partition_broadcast(recip[:], rowA[0:1, rh])
                    nc.vector.tensor_mul(natt[:], unorm[0:DH, :], recip[:])
                    nc.sync.dma_start(attT[m][hp, rh], natt[:])

            emit_v()
            emit_qk_pair(0)
            for m in range(4):
                if m > 0:
                    emit_qk_pair(m)
                # heads jump the priority queue past the v/qk filler blocks:
                # their scores gate ScalarE (the binding engine); v and the
                # next qk pair fill PE gaps instead of blocking exp
                with tc.high_priority(offset=280):
                    emit_head(2 * m)
                with tc.high_priority(offset=280):
                    emit_head(2 * m + 1)

        # ---- Phase 4: out projection ----
        with tc.tile_pool(name="pr_psum", bufs=2, space="PSUM") as pr_psum, \
             tc.tile_pool(name="ostage", bufs=3) as ostage:
            for tt in range(KT):
                ts_ = slice(tt * P, (tt + 1) * P)
                ot = ostage.tile([P, D], fp32, tag="ot", name="ot")
                for c in range(2):
                    cs = slice(c * 512, (c + 1) * 512)
                    ps = pr_psum.tile([P, 512], fp32, tag="pr", name="pr")
                    for m in range(4):
                        nc.tensor.matmul(ps[:], attT[m][:, ts_], wo[m][:, cs],
                                         start=(m == 0), stop=(m == 3))
                    nc.scalar.copy(ot[:, cs], ps[:])
                nc.sync.dma_start(out_d[ts_, :], ot[:])

    nc.compile()
    names = dict(xt=xt_d.name, wqk=wqk_d.name, wv=wv_d.name, wo=wo_d.name,
                 ones=ones_d.name, out=out_d.name)
    return nc, names


def _prep_in_maps(x, ln_g, ln_b, w_qkv, w_out, b_out, names):
    x = np.asarray(x, dtype=np.float32)
    ln_g = np.asarray(ln_g, dtype=np.float64)
    ln_b = np.asarray(ln_b, dtype=np.float64)
    w_qkv = np.asarray(w_qkv, dtype=np.float64)
    w_out = np.asarray(w_out, dtype=np.float32)
    b_out = np.asarray(b_out, dtype=np.float32)

    wq = w_qkv[:, 0 * D:1 * D] * SCALE   # fold softmax scale into q
    wk = w_qkv[:, 1 * D:2 * D]
    wv_ = w_qkv[:, 2 * D:3 * D]

    in_maps = []
    for c in range(NCORES):
        b, g = c // 2, c % 2
        heads = range(g * HL, (g + 1) * HL)
        # wqk cols: [q_h0 q_h1 | ... | q_h6 q_h7 | k_h0 k_h1 | ...] pair tiles
        qcols = np.concatenate([wq[:, h * DH:(h + 1) * DH] for h in heads], axis=1)
        kcols = np.concatenate([wk[:, h * DH:(h + 1) * DH] for h in heads], axis=1)
        vcols = np.concatenate([wv_[:, h * DH:(h + 1) * DH] for h in heads], axis=1)
        wqk_c = np.concatenate([qcols, kcols], axis=1)          # [D, 1024]
        wqk_aug = np.concatenate([
            ln_g[:, None] * wqk_c,
            (ln_g @ wqk_c)[None, :],
            (ln_b @ wqk_c)[None, :]], axis=0)                   # [D+2, 1024]
        wv_aug = np.concatenate([
            ln_g[:, None] * vcols,
            (ln_g @ vcols)[None, :],
            (ln_b @ vcols)[None, :]], axis=0)                   # [D+2, 512]
        wo_c = w_out[g * IN_L:(g + 1) * IN_L, :]                # [512, D]
        in_maps.append({
            names["xt"]: np.ascontiguousarray(x[b].T).astype(BF16),
            names["wqk"]: wqk_aug.astype(np.float32).astype(BF16),
            names["wv"]: wv_aug.astype(np.float32).astype(BF16),
            names["wo"]: np.ascontiguousarray(wo_c).astype(BF16),
            names["ones"]: np.ones((1, N), dtype=BF16),
        })
    return in_maps


def _gather(res, names, b_out):
    out = np.empty((B, N, D), dtype=np.float32)
    for b in range(B):
        out[b] = res.results[2 * b][names["out"]] \
            + res.results[2 * b + 1][names["out"]] + np.asarray(b_out, np.float32)
    return out


def kernel(x, ln_g, ln_b, w_qkv, w_out, b_out):
    global _compiled
    from concourse.bass_utils import run_bass_kernel_spmd

    if _compiled is None:
        _compiled = _build()
    nc, names = _compiled
    in_maps = _prep_in_maps(x, ln_g, ln_b, w_qkv, w_out, b_out, names)
    res = run_bass_kernel_spmd(nc, in_maps, core_ids=list(range(NCORES)))
    return _gather(res, names, b_out)


def run_traced(x, ln_g, ln_b, w_qkv, w_out, b_out):
    """Test helper: run with NTFF tracing enabled, return BassKernelResults."""
    global _compiled
    from concourse.bass_utils import run_bass_kernel_spmd

    if _compiled is None:
        _compiled = _build()
    nc, names = _compiled
    in_maps = _prep_in_maps(x, ln_g, ln_b, w_qkv, w_out, b_out, names)
    return run_bass_kernel_spmd(nc, in_maps, core_ids=list(range(NCORES)),
                                trace=True)


# revision 24
# speedup vs baseline: 1.0054x; 1.0017x over previous
"""Fused LayerNorm + MHA + projections on 8 TRN2 NeuronCores.

Problem (hardcoded): x [4, 2048, 1024] f32, 16 heads x 64 dh, inner 1024.
Sharding: core c = (batch b = c//2, head-group g = c%2, 8 heads each).
Each core returns a partial output [2048, 1024] (its heads' contribution
to the out-projection); host sums the pair per batch and adds b_out.

Per-core math:
  LN folded into QKV:  qkv = rstd*(x @ (g*W)) + (-mu*rstd)*u + r
  (u = sum_f g*W, r = sum_f b*W are host-precomputed aug rows; the rstd
  row is applied at PSUM eviction).
  Attention in the "transposed world": scores^T [keys, q] strips, exp on
  ScalarE (one op per [128, 2048] strip), attn@v with lhsT = [v_h | ones]
  (65 cols) so softmax denominators accumulate in PSUM row 64 for free.
"""

import numpy as np
import ml_dtypes

B, N, D = 4, 2048, 1024
HEADS_TOT, DH = 16, 64
HL = 8               # local heads per core
IN_L = HL * DH       # 512 local inner dim
NCORES = 8
P = 128
KT = N // P          # 16 key tiles
NCH = N // 512       # 4 q chunks of 512
EPS = 1e-5
SCALE = DH ** -0.5

BF16 = ml_dtypes.bfloat16

_compiled = None  # (nc, names) cache


def _build():
    import contextlib
    import concourse.mybir as mybir
    import concourse.tile as tile
    from concourse import bacc

    fp32 = mybir.dt.float32
    bf16 = mybir.dt.bfloat16
    AF = mybir.ActivationFunctionType

    nc = bacc.Bacc("TRN2", target_bir_lowering=False, debug=False)

    with tile.TileContext(nc) as tc, contextlib.ExitStack() as ctx:
        dram = ctx.enter_context(tc.tile_pool(name="dram", bufs=1, space="DRAM"))
        xt_d = dram.tile([D, N], bf16, kind="ExternalInput")        # x[b].T
        wqk_d = dram.tile([D + 2, 2 * IN_L], bf16, kind="ExternalInput")
        wv_d = dram.tile([D + 2, IN_L], bf16, kind="ExternalInput")
        wo_d = dram.tile([IN_L, D], bf16, kind="ExternalInput")
        out_d = dram.tile([N, D], fp32, kind="ExternalOutput")
        ones_d = dram.tile([1, N], bf16, kind="ExternalInput")
        scr_d = dram.tile([1, N], fp32)  # bounce buffer for rstd restripe

        const = ctx.enter_context(tc.tile_pool(name="const", bufs=1))
        # ---- static SBUF tensors ----
        xt = [const.tile([P, N], bf16, tag=f"xt{i}", name=f"xt{i}") for i in range(8)]
        wqk = [const.tile([P, 2 * IN_L], bf16, tag=f"wqk{i}", name=f"wqk{i}") for i in range(8)]
        wqk_aug = const.tile([2, 2 * IN_L], bf16, tag="wqk_aug", name="wqk_aug")
        wv = [const.tile([P, IN_L], bf16, tag=f"wv{i}", name=f"wv{i}") for i in range(8)]
        wv_aug = const.tile([2, IN_L], bf16, tag="wv_aug", name="wv_aug")
        wo = [const.tile([P, D], bf16, tag=f"wo{i}", name=f"wo{i}") for i in range(4)]
        ones_col = const.tile([P, 1], bf16, tag="ones_col", name="ones_col")
        aug = const.tile([2, N], bf16, tag="aug", name="aug")  # [nmr; ones]
        # qkT tiles: 0-3 = q pairs [q_2m|q_2m+1], 4-7 = k pairs
        qkT = [const.tile([P, N], bf16, tag=f"qkT{i}", name=f"qkT{i}") for i in range(8)]
        # v with ones col: head h at cols h*65..h*65+64 (col 64 = ones)
        vt = [const.tile([P, HL * (DH + 1)], bf16, tag=f"vt{i}", name=f"vt{i}")
              for i in range(KT)]
        attT = [const.tile([P, N], bf16, tag=f"attT{i}", name=f"attT{i}") for i in range(4)]
        # [1, N] f32 stat rows; all at base partition 0 (engine ops require
        # matching start partitions across operands)
        rowA = const.tile([1, N], fp32, tag="rowA", name="rowA")  # mu -> sums
        rowB = const.tile([1, N], fp32, tag="rowB", name="rowB")  # sumsq->rstd
        rowC = const.tile([1, N], fp32, tag="rowC", name="rowC")  # scratch
        rstd_b = const.tile([P, N], fp32, tag="rstd_b", name="rstd_b")
        rstd_col = const.tile([P, KT], fp32, tag="rstd_col", name="rstd_col")

        # ---- DMA inputs ----
        for i in range(8):
            nc.sync.dma_start(xt[i][:], xt_d[i * P:(i + 1) * P, :])
        for i in range(8):
            nc.sync.dma_start(wqk[i][:], wqk_d[i * P:(i + 1) * P, :])
        nc.sync.dma_start(wqk_aug[:], wqk_d[D:D + 2, :])
        for i in range(8):
            nc.sync.dma_start(wv[i][:], wv_d[i * P:(i + 1) * P, :])
        nc.sync.dma_start(wv_aug[:], wv_d[D:D + 2, :])
        for i in range(4):
            nc.sync.dma_start(wo[i][:], wo_d[i * P:(i + 1) * P, :])

        nc.vector.memset(ones_col[:], 1.0)
        nc.sync.dma_start(aug[1:2, :], ones_d[:])

        # ---- Phase 1: LN stats ----
        with tc.tile_pool(name="sq", bufs=2) as sq_pool, \
             tc.tile_pool(name="st_psum", bufs=1, space="PSUM") as st_psum:
            ps1 = [st_psum.tile([1, 512], fp32, tag=f"s1{c}", name=f"s1{c}")
                   for c in range(NCH)]
            ps2 = [st_psum.tile([1, 512], fp32, tag=f"s2{c}", name=f"s2{c}")
                   for c in range(NCH)]
            for i in range(8):
                s = sq_pool.tile([P, N], bf16, tag="xsq", name="xsq")
                nc.vector.tensor_mul(s[:], xt[i][:], xt[i][:])
                for c in range(NCH):
                    cs = slice(c * 512, (c + 1) * 512)
                    nc.tensor.matmul(ps1[c][:], ones_col[:], xt[i][:, cs],
                                     start=(i == 0), stop=(i == 7))
                    nc.tensor.matmul(ps2[c][:], ones_col[:], s[:, cs],
                                     start=(i == 0), stop=(i == 7))
            for c in range(NCH):
                cs = slice(c * 512, (c + 1) * 512)
                # fused evict+scale: mu = sum_x/D, ex2 = sum_x2/D
                nc.vector.tensor_scalar_mul(rowA[0:1, cs], ps1[c][:], 1.0 / D)
                nc.vector.tensor_scalar_mul(rowB[0:1, cs], ps2[c][:], 1.0 / D)
        # row math (all [1, N] at base partition 0; different tensors)
        nc.vector.tensor_mul(rowC[:], rowA[:], rowA[:])        # mu^2
        nc.vector.tensor_sub(rowB[:], rowB[:], rowC[:])        # var
        # rstd = exp(-0.5 * ln(var + eps))
        eps_t = const.tile([1, 1], fp32, tag="eps_t", name="eps_t")
        nc.vector.memset(eps_t[:], EPS)
        nc.scalar.activation(rowB[:], rowB[:], AF.Ln, bias=eps_t[:])
        nc.scalar.activation(rowB[:], rowB[:], AF.Exp, scale=-0.5)  # rstd
        # aug row 0 = -mu * rstd (bf16)
        nc.vector.tensor_mul(rowC[:], rowA[:], rowB[:])
        nc.vector.tensor_scalar_mul(aug[0:1, :], rowC[:], -1.0)
        # rstd broadcast down partitions + column-form restripe (DRAM bounce)
        nc.gpsimd.partition_broadcast(rstd_b[:], rowB[:])
        nc.sync.dma_start(scr_d[:], rowB[:])
        nc.sync.dma_start(rstd_col[:],
                          scr_d[:].rearrange("o (t p) -> (o p) t", p=P))

        # ---- Phase 2+3: QKV interleaved with per-head attention ----
        # Emission order: qk pair 0 -> v -> heads 0,1 -> qk pair 1 -> heads
        # 2,3 -> ...  so ScalarE (exp, the binding engine) starts ~4x earlier
        # and PE fills exp-wait gaps with the next pair's QKV matmuls.
        NH = N // 2  # q processed in halves so scores PSUM double-buffers
        with tc.tile_pool(name="qkv_psum", bufs=2, space="PSUM") as qkv_psum, \
             tc.tile_pool(name="s_psum", bufs=2, space="PSUM") as s_psum, \
             tc.tile_pool(name="av_psum", bufs=1, space="PSUM") as av_psum, \
             tc.tile_pool(name="expp", bufs=4) as exp_pool, \
             tc.tile_pool(name="norm", bufs=1) as norm_pool:

            def emit_qk_pair(m):
                for j in (m, 4 + m):
                    js = slice(j * P, (j + 1) * P)
                    for c in range(NCH):
                        cs = slice(c * 512, (c + 1) * 512)
                        ps = qkv_psum.tile([P, 512], fp32, tag="qkv",
                                           name="qkv")
                        for kk in range(8):
                            nc.tensor.matmul(ps[:], wqk[kk][:, js],
                                             xt[kk][:, cs],
                                             start=(kk == 0), stop=False)
                        nc.tensor.matmul(ps[:], wqk_aug[:, js], aug[:, cs],
                                         start=False, stop=True)
                        nc.vector.tensor_mul(qkT[j][:, cs], ps[:],
                                             rstd_b[:, cs])

            def emit_v():
                for tt in range(KT):
                    ts_ = slice(tt * P, (tt + 1) * P)
                    ps = qkv_psum.tile([P, IN_L], fp32, tag="qkv", name="qkv")
                    for kk in range(8):
                        nc.tensor.matmul(ps[:], xt[kk][:, ts_], wv[kk][:],
                                         start=(kk == 0), stop=False)
                    nc.tensor.matmul(ps[:], aug[:, ts_], wv_aug[:],
                                     start=False, stop=True)
                    # evict + rstd scale into strided [128,8,65] view
                    vview = vt[tt][:].rearrange("p (h e) -> p h e", e=DH + 1)
                    nc.vector.tensor_scalar_mul(
                        vview[:, :, 0:DH],
                        ps[:].rearrange("p (h e) -> p h e", e=DH),
                        rstd_col[:, tt:tt + 1])
                    nc.vector.tensor_copy(
                        vview[:, :, DH:DH + 1],
                        ones_col[:, 0:1].rearrange("p (a o) -> p a o", a=1)
                        .broadcast_to((P, HL, 1)))

            def emit_head(h):
                m, half = h // 2, h % 2
                qtile, ktile = qkT[m], qkT[4 + m]
                hp = slice(half * DH, (half + 1) * DH)
                for qh in range(2):
                    qs0 = qh * NH
                    av = av_psum.tile([DH + 1, NH], fp32, tag="av", name="av")
                    for kt in range(KT):
                        ks = slice(kt * P, (kt + 1) * P)
                        sp = s_psum.tile([P, NH], fp32, tag="s", name="s")
                        for c in range(2):
                            nc.tensor.matmul(
                                sp[:, c * 512:(c + 1) * 512], ktile[hp, ks],
                                qtile[hp, qs0 + c * 512:qs0 + (c + 1) * 512],
                                start=True, stop=True)
                        et = exp_pool.tile([P, NH], bf16, tag="exp",
                                           name="exp")
                        nc.scalar.activation(et[:], sp[:], AF.Exp)
                        vl = vt[kt][:, h * (DH + 1):(h + 1) * (DH + 1)]
                        for c in range(2):
                            nc.tensor.matmul(
                                av[:, c * 512:(c + 1) * 512], vl,
                                et[:, c * 512:(c + 1) * 512],
                                start=(kt == 0), stop=(kt == KT - 1))
                    # copy whole av to SBUF immediately so the PSUM bank is
                    # released fast (av pool runs with a single buffer), then
                    # normalize from the SBUF copy: recip of sums row (p64 ->
                    # DMA to p0 -> bcast to 64 partitions) times rows 0..63
                    unorm = norm_pool.tile([DH + 1, NH], fp32, tag="unorm",
                                           name="unorm", bufs=2)
                    recip = norm_pool.tile([DH, NH], fp32, tag="recip",
                                           name="recip")
                    natt = norm_pool.tile([DH, NH], bf16, tag="natt",
                                          name="natt")
                    rh = slice(qs0, qs0 + NH)
                    nc.vector.tensor_copy(unorm[:], av[:])
                    nc.sync.dma_start(rowA[0:1, rh], unorm[DH:DH + 1, :])
                    nc.vector.reciprocal_approx_fast(rowA[0:1, rh],
                                                     rowA[0:1, rh])
                    nc.gpsimd.partition_broadcast(recip[:], rowA[0:1, rh])
                    nc.vector.tensor_mul(natt[:], unorm[0:DH, :], recip[:])
                    nc.sync.dma_start(attT[m][hp, rh], natt[:])

            emit_qk_pair(0)
            emit_v()
            for m in range(1, 4):
                emit_qk_pair(m)
            # heads jump the priority queue: their scores gate ScalarE (the
            # binding engine); v and qk pairs 1-3 fill PE gaps instead of
            # blocking exp
            for h in range(HL):
                with tc.high_priority(offset=1000):
                    emit_head(h)

        # ---- Phase 4: out projection ----
        with tc.tile_pool(name="pr_psum", bufs=2, space="PSUM") as pr_psum, \
             tc.tile_pool(name="ostage", bufs=3) as ostage:
            for tt in range(KT):
                ts_ = slice(tt * P, (tt + 1) * P)
                ot = ostage.tile([P, D], fp32, tag="ot", name="ot")
                for c in range(2):
                    cs = slice(c * 512, (c + 1) * 512)
                    ps = pr_psum.tile([P, 512], fp32, tag="pr", name="pr")
                    for m in range(4):
                        nc.tensor.matmul(ps[:], attT[m][:, ts_], wo[m][:, cs],
                                         start=(m == 0), stop=(m == 3))
                    nc.scalar.copy(ot[:, cs], ps[:])
                nc.sync.dma_start(out_d[ts_, :], ot[:])

    nc.compile()
    names = dict(xt=xt_d.name, wqk=wqk_d.name, wv=wv_d.name, wo=wo_d.name,
                 ones=ones_d.name, out=out_d.name)
    return nc, names


def _prep_in_maps(x, ln_g, ln_b, w_qkv, w_out, b_out, names):
    x = np.asarray(x, dtype=np.float32)
    ln_g = np.asarray(ln_g, dtype=np.float64)
    ln_b = np.asarray(ln_b, dtype=np.float64)
    w_qkv = np.asarray(w_qkv, dtype=np.float64)
    w_out = np.asarray(w_out, dtype=np.float32)
    b_out = np.asarray(b_out, dtype=np.float32)

    wq = w_qkv[:, 0 * D:1 * D] * SCALE   # fold softmax scale into q
    wk = w_qkv[:, 1 * D:2 * D]
    wv_ = w_qkv[:, 2 * D:3 * D]

    in_maps = []
    for c in range(NCORES):
        b, g = c // 2, c % 2
        heads = range(g * HL, (g + 1) * HL)
        # wqk cols: [q_h0 q_h1 | ... | q_h6 q_h7 | k_h0 k_h1 | ...] pair tiles
        qcols = np.concatenate([wq[:, h * DH:(h + 1) * DH] for h in heads], axis=1)
        kcols = np.concatenate([wk[:, h * DH:(h + 1) * DH] for h in heads], axis=1)
        vcols = np.concatenate([wv_[:, h * DH:(h + 1) * DH] for h in heads], axis=1)
        wqk_c = np.concatenate([qcols, kcols], axis=1)          # [D, 1024]
        wqk_aug = np.concatenate([
            ln_g[:, None] * wqk_c,
            (ln_g @ wqk_c)[None, :],
            (ln_b @ wqk_c)[None, :]], axis=0)                   # [D+2, 1024]
        wv_aug = np.concatenate([
            ln_g[:, None] * vcols,
            (ln_g @ vcols)[None, :],
            (ln_b @ vcols)[None, :]], axis=0)                   # [D+2, 512]
        wo_c = w_out[g * IN_L:(g + 1) * IN_L, :]                # [512, D]
        in_maps.append({
            names["xt"]: np.ascontiguousarray(x[b].T).astype(BF16),
            names["wqk"]: wqk_aug.astype(np.float32).astype(BF16),
            names["wv"]: wv_aug.astype(np.float32).astype(BF16),
            names["wo"]: np.ascontiguousarray(wo_c).astype(BF16),
            names["ones"]: np.ones((1, N), dtype=BF16),
        })
    return in_maps


def _gather(res, names, b_out):
    out = np.empty((B, N, D), dtype=np.float32)
    for b in range(B):
        out[b] = res.results[2 * b][names["out"]] \
            + res.results[2 * b + 1][names["out"]] + np.asarray(b_out, np.float32)
    return out


def kernel(x, ln_g, ln_b, w_qkv, w_out, b_out):
    global _compiled
    from concourse.bass_utils import run_bass_kernel_spmd

    if _compiled is None:
        _compiled = _build()
    nc, names = _compiled
    in_maps = _prep_in_maps(x, ln_g, ln_b, w_qkv, w_out, b_out, names)
    res = run_bass_kernel_spmd(nc, in_maps, core_ids=list(range(NCORES)))
    return _gather(res, names, b_out)


def run_traced(x, ln_g, ln_b, w_qkv, w_out, b_out):
    """Test helper: run with NTFF tracing enabled, return BassKernelResults."""
    global _compiled
    from concourse.bass_utils import run_bass_kernel_spmd

    if _compiled is None:
        _compiled = _build()
    nc, names = _compiled
    in_maps = _prep_in_maps(x, ln_g, ln_b, w_qkv, w_out, b_out, names)
    return run_bass_kernel_spmd(nc, in_maps, core_ids=list(range(NCORES)),
                                trace=True)


# revision 26
# speedup vs baseline: 1.0210x; 1.0155x over previous
"""Fused LayerNorm + MHA + projections on 8 TRN2 NeuronCores.

Problem (hardcoded): x [4, 2048, 1024] f32, 16 heads x 64 dh, inner 1024.
Sharding: core c = (batch b = c//2, head-group g = c%2, 8 heads each).
Each core returns a partial output [2048, 1024] (its heads' contribution
to the out-projection); host sums the pair per batch and adds b_out.

Per-core math:
  LN folded into QKV:  qkv = rstd*(x @ (g*W)) + (-mu*rstd)*u + r
  (u = sum_f g*W, r = sum_f b*W are host-precomputed aug rows; the rstd
  row is applied at PSUM eviction).
  Attention in the "transposed world": scores^T [keys, q] strips, exp on
  ScalarE (one op per [128, 2048] strip), attn@v with lhsT = [v_h | ones]
  (65 cols) so softmax denominators accumulate in PSUM row 64 for free.
"""

import numpy as np
import ml_dtypes

B, N, D = 4, 2048, 1024
HEADS_TOT, DH = 16, 64
HL = 8               # local heads per core
IN_L = HL * DH       # 512 local inner dim
NCORES = 8
P = 128
KT = N // P          # 16 key tiles
NCH = N // 512       # 4 q chunks of 512
EPS = 1e-5
SCALE = DH ** -0.5

BF16 = ml_dtypes.bfloat16

_compiled = None  # (nc, names) cache


def _build():
    import contextlib
    import concourse.mybir as mybir
    import concourse.tile as tile
    from concourse import bacc

    fp32 = mybir.dt.float32
    bf16 = mybir.dt.bfloat16
    AF = mybir.ActivationFunctionType

    nc = bacc.Bacc("TRN2", target_bir_lowering=False, debug=False)

    with tile.TileContext(nc) as tc, contextlib.ExitStack() as ctx:
        dram = ctx.enter_context(tc.tile_pool(name="dram", bufs=1, space="DRAM"))
        xt_d = dram.tile([D, N], bf16, kind="ExternalInput")        # x[b].T
        wqk_d = dram.tile([D + 2, 2 * IN_L], bf16, kind="ExternalInput")
        wv_d = dram.tile([D + 2, IN_L], bf16, kind="ExternalInput")
        wo_d = dram.tile([IN_L, D], bf16, kind="ExternalInput")
        out_d = dram.tile([N, D], fp32, kind="ExternalOutput")
        ones_d = dram.tile([1, N], bf16, kind="ExternalInput")
        scr_d = dram.tile([1, N], fp32)  # bounce buffer for rstd restripe

        const = ctx.enter_context(tc.tile_pool(name="const", bufs=1))
        # ---- static SBUF tensors ----
        xt = [const.tile([P, N], bf16, tag=f"xt{i}", name=f"xt{i}") for i in range(8)]
        wqk = [const.tile([P, 2 * IN_L], bf16, tag=f"wqk{i}", name=f"wqk{i}") for i in range(8)]
        wqk_aug = const.tile([2, 2 * IN_L], bf16, tag="wqk_aug", name="wqk_aug")
        wv = [const.tile([P, IN_L], bf16, tag=f"wv{i}", name=f"wv{i}") for i in range(8)]
        wv_aug = const.tile([2, IN_L], bf16, tag="wv_aug", name="wv_aug")
        wo = [const.tile([P, D], bf16, tag=f"wo{i}", name=f"wo{i}") for i in range(4)]
        ones_col = const.tile([P, 1], bf16, tag="ones_col", name="ones_col")
        aug = const.tile([2, N], bf16, tag="aug", name="aug")  # [nmr; ones]
        # qkT tiles: 0-3 = q pairs [q_2m|q_2m+1], 4-7 = k pairs
        qkT = [const.tile([P, N], bf16, tag=f"qkT{i}", name=f"qkT{i}") for i in range(8)]
        # v with ones col: head h at cols h*65..h*65+64 (col 64 = ones)
        vt = [const.tile([P, HL * (DH + 1)], bf16, tag=f"vt{i}", name=f"vt{i}")
              for i in range(KT)]
        attT = [const.tile([P, N], bf16, tag=f"attT{i}", name=f"attT{i}") for i in range(4)]
        # [1, N] f32 stat rows; all at base partition 0 (engine ops require
        # matching start partitions across operands)
        rowA = const.tile([1, N], fp32, tag="rowA", name="rowA")  # mu -> sums
        rstd_b = const.tile([P, N], bf16, tag="rstd_b", name="rstd_b")
        rstd_col = const.tile([P, KT], fp32, tag="rstd_col", name="rstd_col")

        # ---- DMA inputs ----
        for i in range(8):
            nc.sync.dma_start(xt[i][:], xt_d[i * P:(i + 1) * P, :])
        for i in range(8):
            nc.sync.dma_start(wqk[i][:], wqk_d[i * P:(i + 1) * P, :])
        nc.sync.dma_start(wqk_aug[:], wqk_d[D:D + 2, :])
        for i in range(8):
            nc.sync.dma_start(wv[i][:], wv_d[i * P:(i + 1) * P, :])
        nc.sync.dma_start(wv_aug[:], wv_d[D:D + 2, :])
        for i in range(4):
            nc.sync.dma_start(wo[i][:], wo_d[i * P:(i + 1) * P, :])

        nc.vector.memset(ones_col[:], 1.0)
        nc.sync.dma_start(aug[1:2, :], ones_d[:])

        # ---- Phase 1: LN stats ----
        with tc.tile_pool(name="sq", bufs=2) as sq_pool, \
             tc.tile_pool(name="strow", bufs=1) as strow_pool, \
             tc.tile_pool(name="st_psum", bufs=1, space="PSUM") as st_psum:
            rowB = strow_pool.tile([1, N], fp32, tag="rowB", name="rowB")
            rowC = strow_pool.tile([1, N], fp32, tag="rowC", name="rowC")
            rowBh = strow_pool.tile([1, N], bf16, tag="rowBh", name="rowBh")
            ps1 = [st_psum.tile([1, 512], fp32, tag=f"s1{c}", name=f"s1{c}")
                   for c in range(NCH)]
            ps2 = [st_psum.tile([1, 512], fp32, tag=f"s2{c}", name=f"s2{c}")
                   for c in range(NCH)]
            for i in range(8):
                s = sq_pool.tile([P, N], bf16, tag="xsq", name="xsq")
                nc.vector.tensor_mul(s[:], xt[i][:], xt[i][:])
                for c in range(NCH):
                    cs = slice(c * 512, (c + 1) * 512)
                    nc.tensor.matmul(ps1[c][:], ones_col[:], xt[i][:, cs],
                                     start=(i == 0), stop=(i == 7))
                    nc.tensor.matmul(ps2[c][:], ones_col[:], s[:, cs],
                                     start=(i == 0), stop=(i == 7))
            for c in range(NCH):
                cs = slice(c * 512, (c + 1) * 512)
                # fused evict+scale: mu = sum_x/D, ex2 = sum_x2/D
                nc.vector.tensor_scalar_mul(rowA[0:1, cs], ps1[c][:], 1.0 / D)
                nc.vector.tensor_scalar_mul(rowB[0:1, cs], ps2[c][:], 1.0 / D)
            # row math (all [1, N] at base partition 0; different tensors)
            nc.vector.tensor_mul(rowC[:], rowA[:], rowA[:])        # mu^2
            nc.vector.tensor_sub(rowB[:], rowB[:], rowC[:])        # var
            # rstd = exp(-0.5 * ln(var + eps))
            eps_t = const.tile([1, 1], fp32, tag="eps_t", name="eps_t")
            nc.vector.memset(eps_t[:], EPS)
            nc.scalar.activation(rowB[:], rowB[:], AF.Ln, bias=eps_t[:])
            nc.scalar.activation(rowB[:], rowB[:], AF.Exp, scale=-0.5)
            # aug row 0 = -mu * rstd (bf16)
            nc.vector.tensor_mul(rowC[:], rowA[:], rowB[:])
            nc.vector.tensor_scalar_mul(aug[0:1, :], rowC[:], -1.0)
            # rstd broadcast down partitions + column restripe (DRAM bounce)
            nc.vector.tensor_copy(rowBh[:], rowB[:])
            nc.gpsimd.partition_broadcast(rstd_b[:], rowBh[:])
            nc.sync.dma_start(scr_d[:], rowB[:])
            nc.sync.dma_start(rstd_col[:],
                              scr_d[:].rearrange("o (t p) -> (o p) t", p=P))

        # ---- Phase 2+3: QKV interleaved with per-head attention ----
        # Emission order: qk pair 0 -> v -> heads 0,1 -> qk pair 1 -> heads
        # 2,3 -> ...  so ScalarE (exp, the binding engine) starts ~4x earlier
        # and PE fills exp-wait gaps with the next pair's QKV matmuls.
        NH = N // 2  # q processed in halves so scores PSUM double-buffers
        with tc.tile_pool(name="qkv_psum", bufs=2, space="PSUM") as qkv_psum, \
             tc.tile_pool(name="s_psum", bufs=2, space="PSUM") as s_psum, \
             tc.tile_pool(name="av_psum", bufs=1, space="PSUM") as av_psum, \
             tc.tile_pool(name="expp", bufs=12) as exp_pool, \
             tc.tile_pool(name="norm", bufs=1) as norm_pool:

            def emit_qk_pair(m):
                for j in (m, 4 + m):
                    js = slice(j * P, (j + 1) * P)
                    for c in range(NCH):
                        cs = slice(c * 512, (c + 1) * 512)
                        ps = qkv_psum.tile([P, 512], fp32, tag="qkv",
                                           name="qkv")
                        for kk in range(8):
                            nc.tensor.matmul(ps[:], wqk[kk][:, js],
                                             xt[kk][:, cs],
                                             start=(kk == 0), stop=False)
                        nc.tensor.matmul(ps[:], wqk_aug[:, js], aug[:, cs],
                                         start=False, stop=True)
                        nc.vector.tensor_mul(qkT[j][:, cs], ps[:],
                                             rstd_b[:, cs])

            def emit_v():
                for tt in range(KT):
                    ts_ = slice(tt * P, (tt + 1) * P)
                    ps = qkv_psum.tile([P, IN_L], fp32, tag="qkv", name="qkv")
                    for kk in range(8):
                        nc.tensor.matmul(ps[:], xt[kk][:, ts_], wv[kk][:],
                                         start=(kk == 0), stop=False)
                    nc.tensor.matmul(ps[:], aug[:, ts_], wv_aug[:],
                                     start=False, stop=True)
                    # evict + rstd scale into strided [128,8,65] view
                    vview = vt[tt][:].rearrange("p (h e) -> p h e", e=DH + 1)
                    nc.vector.tensor_scalar_mul(
                        vview[:, :, 0:DH],
                        ps[:].rearrange("p (h e) -> p h e", e=DH),
                        rstd_col[:, tt:tt + 1])
                    nc.vector.tensor_copy(
                        vview[:, :, DH:DH + 1],
                        ones_col[:, 0:1].rearrange("p (a o) -> p a o", a=1)
                        .broadcast_to((P, HL, 1)))

            def emit_head(h):
                m, half = h // 2, h % 2
                qtile, ktile = qkT[m], qkT[4 + m]
                hp = slice(half * DH, (half + 1) * DH)
                for qh in range(2):
                    qs0 = qh * NH
                    av = av_psum.tile([DH + 1, NH], fp32, tag="av", name="av")
                    for kt in range(KT):
                        ks = slice(kt * P, (kt + 1) * P)
                        sp = s_psum.tile([P, NH], fp32, tag="s", name="s")
                        for c in range(2):
                            nc.tensor.matmul(
                                sp[:, c * 512:(c + 1) * 512], ktile[hp, ks],
                                qtile[hp, qs0 + c * 512:qs0 + (c + 1) * 512],
                                start=True, stop=True)
                        et = exp_pool.tile([P, NH], bf16, tag="exp",
                                           name="exp")
                        nc.scalar.activation(et[:], sp[:], AF.Exp)
                        vl = vt[kt][:, h * (DH + 1):(h + 1) * (DH + 1)]
                        for c in range(2):
                            nc.tensor.matmul(
                                av[:, c * 512:(c + 1) * 512], vl,
                                et[:, c * 512:(c + 1) * 512],
                                start=(kt == 0), stop=(kt == KT - 1))
                    # copy whole av to SBUF immediately so the PSUM bank is
                    # released fast (av pool runs with a single buffer), then
                    # normalize from the SBUF copy: recip of sums row (p64 ->
                    # DMA to p0 -> bcast to 64 partitions) times rows 0..63
                    unorm = norm_pool.tile([DH + 1, NH], fp32, tag="unorm",
                                           name="unorm", bufs=3)
                    recip = norm_pool.tile([DH, NH], fp32, tag="recip",
                                           name="recip", bufs=2)
                    natt = norm_pool.tile([DH, NH], bf16, tag="natt",
                                          name="natt", bufs=2)
                    rh = slice(qs0, qs0 + NH)
                    nc.vector.tensor_copy(unorm[:], av[:])
                    nc.sync.dma_start(rowA[0:1, rh], unorm[DH:DH + 1, :])
                    nc.vector.reciprocal_approx_fast(rowA[0:1, rh],
                                                     rowA[0:1, rh])
                    nc.gpsimd.partition_broadcast(recip[:], rowA[0:1, rh])
                    nc.vector.tensor_mul(natt[:], unorm[0:DH, :], recip[:])
                    nc.sync.dma_start(attT[m][hp, rh], natt[:])

            emit_qk_pair(0)
            emit_v()
            for m in range(1, 4):
                emit_qk_pair(m)
            # heads jump the priority queue: their scores gate ScalarE (the
            # binding engine); v and qk pairs 1-3 fill PE gaps instead of
            # blocking exp
            for h in range(HL):
                with tc.high_priority(offset=1000):
                    emit_head(h)

        # ---- Phase 4: out projection ----
        with tc.tile_pool(name="pr_psum", bufs=2, space="PSUM") as pr_psum, \
             tc.tile_pool(name="ostage", bufs=3) as ostage:
            for tt in range(KT):
                ts_ = slice(tt * P, (tt + 1) * P)
                ot = ostage.tile([P, D], fp32, tag="ot", name="ot")
                for c in range(2):
                    cs = slice(c * 512, (c + 1) * 512)
                    ps = pr_psum.tile([P, 512], fp32, tag="pr", name="pr")
                    for m in range(4):
                        nc.tensor.matmul(ps[:], attT[m][:, ts_], wo[m][:, cs],
                                         start=(m == 0), stop=(m == 3))
                    nc.scalar.copy(ot[:, cs], ps[:])
                nc.sync.dma_start(out_d[ts_, :], ot[:])

    nc.compile()
    names = dict(xt=xt_d.name, wqk=wqk_d.name, wv=wv_d.name, wo=wo_d.name,
                 ones=ones_d.name, out=out_d.name)
    return nc, names


def _prep_in_maps(x, ln_g, ln_b, w_qkv, w_out, b_out, names):
    x = np.asarray(x, dtype=np.float32)
    ln_g = np.asarray(ln_g, dtype=np.float64)
    ln_b = np.asarray(ln_b, dtype=np.float64)
    w_qkv = np.asarray(w_qkv, dtype=np.float64)
    w_out = np.asarray(w_out, dtype=np.float32)
    b_out = np.asarray(b_out, dtype=np.float32)

    wq = w_qkv[:, 0 * D:1 * D] * SCALE   # fold softmax scale into q
    wk = w_qkv[:, 1 * D:2 * D]
    wv_ = w_qkv[:, 2 * D:3 * D]

    in_maps = []
    for c in range(NCORES):
        b, g = c // 2, c % 2
        heads = range(g * HL, (g + 1) * HL)
        # wqk cols: [q_h0 q_h1 | ... | q_h6 q_h7 | k_h0 k_h1 | ...] pair tiles
        qcols = np.concatenate([wq[:, h * DH:(h + 1) * DH] for h in heads], axis=1)
        kcols = np.concatenate([wk[:, h * DH:(h + 1) * DH] for h in heads], axis=1)
        vcols = np.concatenate([wv_[:, h * DH:(h + 1) * DH] for h in heads], axis=1)
        wqk_c = np.concatenate([qcols, kcols], axis=1)          # [D, 1024]
        wqk_aug = np.concatenate([
            ln_g[:, None] * wqk_c,
            (ln_g @ wqk_c)[None, :],
            (ln_b @ wqk_c)[None, :]], axis=0)                   # [D+2, 1024]
        wv_aug = np.concatenate([
            ln_g[:, None] * vcols,
            (ln_g @ vcols)[None, :],
            (ln_b @ vcols)[None, :]], axis=0)                   # [D+2, 512]
        wo_c = w_out[g * IN_L:(g + 1) * IN_L, :]                # [512, D]
        in_maps.append({
            names["xt"]: np.ascontiguousarray(x[b].T).astype(BF16),
            names["wqk"]: wqk_aug.astype(np.float32).astype(BF16),
            names["wv"]: wv_aug.astype(np.float32).astype(BF16),
            names["wo"]: np.ascontiguousarray(wo_c).astype(BF16),
            names["ones"]: np.ones((1, N), dtype=BF16),
        })
    return in_maps


def _gather(res, names, b_out):
    out = np.empty((B, N, D), dtype=np.float32)
    for b in range(B):
        out[b] = res.results[2 * b][names["out"]] \
            + res.results[2 * b + 1][names["out"]] + np.asarray(b_out, np.float32)
    return out


def kernel(x, ln_g, ln_b, w_qkv, w_out, b_out):
    global _compiled
    from concourse.bass_utils import run_bass_kernel_spmd

    if _compiled is None:
        _compiled = _build()
    nc, names = _compiled
    in_maps = _prep_in_maps(x, ln_g, ln_b, w_qkv, w_out, b_out, names)
    res = run_bass_kernel_spmd(nc, in_maps, core_ids=list(range(NCORES)))
    return _gather(res, names, b_out)


def run_traced(x, ln_g, ln_b, w_qkv, w_out, b_out):
    """Test helper: run with NTFF tracing enabled, return BassKernelResults."""
    global _compiled
    from concourse.bass_utils import run_bass_kernel_spmd

    if _compiled is None:
        _compiled = _build()
    nc, names = _compiled
    in_maps = _prep_in_maps(x, ln_g, ln_b, w_qkv, w_out, b_out, names)
    return run_bass_kernel_spmd(nc, in_maps, core_ids=list(range(NCORES)),
                                trace=True)


# revision 28
# speedup vs baseline: 1.0289x; 1.0077x over previous
"""Fused LayerNorm + MHA + projections on 8 TRN2 NeuronCores.

Problem (hardcoded): x [4, 2048, 1024] f32, 16 heads x 64 dh, inner 1024.
Sharding: core c = (batch b = c//2, head-group g = c%2, 8 heads each).
Each core returns a partial output [2048, 1024] (its heads' contribution
to the out-projection); host sums the pair per batch and adds b_out.

Per-core math:
  LN folded into QKV:  qkv = rstd*(x @ (g*W)) + (-mu*rstd)*u + r
  (u = sum_f g*W, r = sum_f b*W are host-precomputed aug rows; the rstd
  row is applied at PSUM eviction).
  Attention in the "transposed world": scores^T [keys, q] strips, exp on
  ScalarE (one op per [128, 2048] strip), attn@v with lhsT = [v_h | ones]
  (65 cols) so softmax denominators accumulate in PSUM row 64 for free.
"""

import numpy as np
import ml_dtypes

B, N, D = 4, 2048, 1024
HEADS_TOT, DH = 16, 64
HL = 8               # local heads per core
IN_L = HL * DH       # 512 local inner dim
NCORES = 8
P = 128
KT = N // P          # 16 key tiles
NCH = N // 512       # 4 q chunks of 512
EPS = 1e-5
SCALE = DH ** -0.5

BF16 = ml_dtypes.bfloat16

_compiled = None  # (nc, names) cache


def _build():
    import contextlib
    import concourse.mybir as mybir
    import concourse.tile as tile
    from concourse import bacc

    fp32 = mybir.dt.float32
    bf16 = mybir.dt.bfloat16
    AF = mybir.ActivationFunctionType

    nc = bacc.Bacc("TRN2", target_bir_lowering=False, debug=False)

    with tile.TileContext(nc) as tc, contextlib.ExitStack() as ctx:
        dram = ctx.enter_context(tc.tile_pool(name="dram", bufs=1, space="DRAM"))
        xt_d = dram.tile([D, N], bf16, kind="ExternalInput")        # x[b].T
        wqk_d = dram.tile([D + 2, 2 * IN_L], bf16, kind="ExternalInput")
        wv_d = dram.tile([D + 2, IN_L], bf16, kind="ExternalInput")
        wo_d = dram.tile([IN_L, D], bf16, kind="ExternalInput")
        out_d = dram.tile([N, D], fp32, kind="ExternalOutput")
        ones_d = dram.tile([1, N], bf16, kind="ExternalInput")
        scr_d = dram.tile([1, N], fp32)  # bounce buffer for rstd restripe

        const = ctx.enter_context(tc.tile_pool(name="const", bufs=1))
        # ---- static SBUF tensors ----
        xt = [const.tile([P, N], bf16, tag=f"xt{i}", name=f"xt{i}") for i in range(8)]
        wqk = [const.tile([P, 2 * IN_L], bf16, tag=f"wqk{i}", name=f"wqk{i}") for i in range(8)]
        wqk_aug = const.tile([2, 2 * IN_L], bf16, tag="wqk_aug", name="wqk_aug")
        wv = [const.tile([P, IN_L], bf16, tag=f"wv{i}", name=f"wv{i}") for i in range(8)]
        wv_aug = const.tile([2, IN_L], bf16, tag="wv_aug", name="wv_aug")
        wo = [const.tile([P, D], bf16, tag=f"wo{i}", name=f"wo{i}") for i in range(4)]
        ones_col = const.tile([P, 1], bf16, tag="ones_col", name="ones_col")
        aug = const.tile([2, N], bf16, tag="aug", name="aug")  # [nmr; ones]
        # qkT tiles: 0-3 = q pairs [q_2m|q_2m+1], 4-7 = k pairs
        qkT = [const.tile([P, N], bf16, tag=f"qkT{i}", name=f"qkT{i}") for i in range(8)]
        # v with ones col: head h at cols h*65..h*65+64 (col 64 = ones)
        vt = [const.tile([P, HL * (DH + 1)], bf16, tag=f"vt{i}", name=f"vt{i}")
              for i in range(KT)]
        attT = [const.tile([P, N], bf16, tag=f"attT{i}", name=f"attT{i}") for i in range(4)]
        # [1, N] f32 stat rows; all at base partition 0 (engine ops require
        # matching start partitions across operands)
        rowA = const.tile([1, N], fp32, tag="rowA", name="rowA")  # mu -> sums
        rstd_b = const.tile([P, N], bf16, tag="rstd_b", name="rstd_b")
        rstd_col = const.tile([P, KT], fp32, tag="rstd_col", name="rstd_col")

        # ---- DMA inputs ----
        for i in range(8):
            nc.sync.dma_start(xt[i][:], xt_d[i * P:(i + 1) * P, :])
        for i in range(8):
            nc.sync.dma_start(wqk[i][:], wqk_d[i * P:(i + 1) * P, :])
        nc.sync.dma_start(wqk_aug[:], wqk_d[D:D + 2, :])
        for i in range(8):
            nc.sync.dma_start(wv[i][:], wv_d[i * P:(i + 1) * P, :])
        nc.sync.dma_start(wv_aug[:], wv_d[D:D + 2, :])
        for i in range(4):
            nc.sync.dma_start(wo[i][:], wo_d[i * P:(i + 1) * P, :])

        nc.vector.memset(ones_col[:], 1.0)
        nc.sync.dma_start(aug[1:2, :], ones_d[:])

        # ---- Phase 1: LN stats ----
        with tc.tile_pool(name="sq", bufs=2) as sq_pool, \
             tc.tile_pool(name="strow", bufs=1) as strow_pool, \
             tc.tile_pool(name="st_psum", bufs=1, space="PSUM") as st_psum:
            rowB = strow_pool.tile([1, N], fp32, tag="rowB", name="rowB")
            rowC = strow_pool.tile([1, N], fp32, tag="rowC", name="rowC")
            rowBh = strow_pool.tile([1, N], bf16, tag="rowBh", name="rowBh")
            ps1 = [st_psum.tile([1, 512], fp32, tag=f"s1{c}", name=f"s1{c}")
                   for c in range(NCH)]
            ps2 = [st_psum.tile([1, 512], fp32, tag=f"s2{c}", name=f"s2{c}")
                   for c in range(NCH)]
            for i in range(8):
                s = sq_pool.tile([P, N], bf16, tag="xsq", name="xsq")
                nc.vector.tensor_mul(s[:], xt[i][:], xt[i][:])
                for c in range(NCH):
                    cs = slice(c * 512, (c + 1) * 512)
                    nc.tensor.matmul(ps1[c][:], ones_col[:], xt[i][:, cs],
                                     start=(i == 0), stop=(i == 7))
                    nc.tensor.matmul(ps2[c][:], ones_col[:], s[:, cs],
                                     start=(i == 0), stop=(i == 7))
            for c in range(NCH):
                cs = slice(c * 512, (c + 1) * 512)
                # fused evict+scale: mu = sum_x/D, ex2 = sum_x2/D
                nc.vector.tensor_scalar_mul(rowA[0:1, cs], ps1[c][:], 1.0 / D)
                nc.vector.tensor_scalar_mul(rowB[0:1, cs], ps2[c][:], 1.0 / D)
            # row math (all [1, N] at base partition 0; different tensors)
            nc.vector.tensor_mul(rowC[:], rowA[:], rowA[:])        # mu^2
            nc.vector.tensor_sub(rowB[:], rowB[:], rowC[:])        # var
            # rstd = exp(-0.5 * ln(var + eps))
            eps_t = const.tile([1, 1], fp32, tag="eps_t", name="eps_t")
            nc.vector.memset(eps_t[:], EPS)
            nc.scalar.activation(rowB[:], rowB[:], AF.Ln, bias=eps_t[:])
            nc.scalar.activation(rowB[:], rowB[:], AF.Exp, scale=-0.5)
            # aug row 0 = -mu * rstd (bf16)
            nc.vector.tensor_mul(rowC[:], rowA[:], rowB[:])
            nc.vector.tensor_scalar_mul(aug[0:1, :], rowC[:], -1.0)
            # rstd broadcast down partitions + column restripe (DRAM bounce)
            nc.vector.tensor_copy(rowBh[:], rowB[:])
            nc.gpsimd.partition_broadcast(rstd_b[:], rowBh[:])
            nc.sync.dma_start(scr_d[:], rowB[:])
            nc.sync.dma_start(rstd_col[:],
                              scr_d[:].rearrange("o (t p) -> (o p) t", p=P))

        # ---- Phase 2+3: QKV interleaved with per-head attention ----
        # Emission order: qk pair 0 -> v -> heads 0,1 -> qk pair 1 -> heads
        # 2,3 -> ...  so ScalarE (exp, the binding engine) starts ~4x earlier
        # and PE fills exp-wait gaps with the next pair's QKV matmuls.
        NH = N // 2  # q processed in halves so scores PSUM double-buffers
        with tc.tile_pool(name="qkv_psum", bufs=2, space="PSUM") as qkv_psum, \
             tc.tile_pool(name="s_psum", bufs=2, space="PSUM") as s_psum, \
             tc.tile_pool(name="av_psum", bufs=1, space="PSUM") as av_psum, \
             tc.tile_pool(name="expp", bufs=12) as exp_pool, \
             tc.tile_pool(name="norm", bufs=1) as norm_pool:

            def emit_qk_pair(m):
                for j in (m, 4 + m):
                    js = slice(j * P, (j + 1) * P)
                    for c in range(NCH):
                        cs = slice(c * 512, (c + 1) * 512)
                        ps = qkv_psum.tile([P, 512], fp32, tag="qkv",
                                           name="qkv")
                        for kk in range(8):
                            nc.tensor.matmul(ps[:], wqk[kk][:, js],
                                             xt[kk][:, cs],
                                             start=(kk == 0), stop=False)
                        nc.tensor.matmul(ps[:], wqk_aug[:, js], aug[:, cs],
                                         start=False, stop=True)
                        nc.vector.tensor_mul(qkT[j][:, cs], ps[:],
                                             rstd_b[:, cs])

            def emit_v():
                for tt in range(KT):
                    ts_ = slice(tt * P, (tt + 1) * P)
                    ps = qkv_psum.tile([P, IN_L], fp32, tag="qkv", name="qkv")
                    for kk in range(8):
                        nc.tensor.matmul(ps[:], xt[kk][:, ts_], wv[kk][:],
                                         start=(kk == 0), stop=False)
                    nc.tensor.matmul(ps[:], aug[:, ts_], wv_aug[:],
                                     start=False, stop=True)
                    # evict + rstd scale into strided [128,8,65] view
                    vview = vt[tt][:].rearrange("p (h e) -> p h e", e=DH + 1)
                    nc.vector.tensor_scalar_mul(
                        vview[:, :, 0:DH],
                        ps[:].rearrange("p (h e) -> p h e", e=DH),
                        rstd_col[:, tt:tt + 1])
                    nc.vector.tensor_copy(
                        vview[:, :, DH:DH + 1],
                        ones_col[:, 0:1].rearrange("p (a o) -> p a o", a=1)
                        .broadcast_to((P, HL, 1)))

            def emit_head(h):
                m, half = h // 2, h % 2
                qtile, ktile = qkT[m], qkT[4 + m]
                hp = slice(half * DH, (half + 1) * DH)
                for qh in range(2):
                    qs0 = qh * NH
                    av = av_psum.tile([DH + 1, NH], fp32, tag="av", name="av")
                    ets = {}

                    def emit_av(kt):
                        vl = vt[kt][:, h * (DH + 1):(h + 1) * (DH + 1)]
                        et = ets.pop(kt)
                        for c in range(2):
                            nc.tensor.matmul(
                                av[:, c * 512:(c + 1) * 512], vl,
                                et[:, c * 512:(c + 1) * 512],
                                start=(kt == 0), stop=(kt == KT - 1))

                    for kt in range(KT):
                        ks = slice(kt * P, (kt + 1) * P)
                        sp = s_psum.tile([P, NH], fp32, tag="s", name="s")
                        for c in range(2):
                            nc.tensor.matmul(
                                sp[:, c * 512:(c + 1) * 512], ktile[hp, ks],
                                qtile[hp, qs0 + c * 512:qs0 + (c + 1) * 512],
                                start=True, stop=True)
                        et = exp_pool.tile([P, NH], bf16, tag="exp",
                                           name="exp")
                        nc.scalar.activation(et[:], sp[:], AF.Exp)
                        ets[kt] = et
                        # av lags one strip so its in-stream wait on exp[kt-1]
                        # is already satisfied (engine streams are in-order)
                        if kt > 0:
                            emit_av(kt - 1)
                    emit_av(KT - 1)
                    # copy whole av to SBUF immediately so the PSUM bank is
                    # released fast (av pool runs with a single buffer), then
                    # normalize from the SBUF copy: recip of sums row (p64 ->
                    # DMA to p0 -> bcast to 64 partitions) times rows 0..63
                    unorm = norm_pool.tile([DH + 1, NH], fp32, tag="unorm",
                                           name="unorm", bufs=3)
                    recip = norm_pool.tile([DH, NH], fp32, tag="recip",
                                           name="recip", bufs=2)
                    natt = norm_pool.tile([DH, NH], bf16, tag="natt",
                                          name="natt", bufs=2)
                    rh = slice(qs0, qs0 + NH)
                    nc.vector.tensor_copy(unorm[:], av[:])
                    nc.sync.dma_start(rowA[0:1, rh], unorm[DH:DH + 1, :])
                    nc.vector.reciprocal_approx_fast(rowA[0:1, rh],
                                                     rowA[0:1, rh])
                    nc.gpsimd.partition_broadcast(recip[:], rowA[0:1, rh])
                    nc.vector.tensor_mul(natt[:], unorm[0:DH, :], recip[:])
                    nc.sync.dma_start(attT[m][hp, rh], natt[:])

            emit_qk_pair(0)
            emit_v()
            for m in range(1, 4):
                emit_qk_pair(m)
            # heads jump the priority queue: their scores gate ScalarE (the
            # binding engine); v and qk pairs 1-3 fill PE gaps instead of
            # blocking exp
            for h in range(HL):
                with tc.high_priority(offset=1000):
                    emit_head(h)

        # ---- Phase 4: out projection ----
        with tc.tile_pool(name="pr_psum", bufs=2, space="PSUM") as pr_psum, \
             tc.tile_pool(name="ostage", bufs=3) as ostage:
            for tt in range(KT):
                ts_ = slice(tt * P, (tt + 1) * P)
                ot = ostage.tile([P, D], fp32, tag="ot", name="ot")
                for c in range(2):
                    cs = slice(c * 512, (c + 1) * 512)
                    ps = pr_psum.tile([P, 512], fp32, tag="pr", name="pr")
                    for m in range(4):
                        nc.tensor.matmul(ps[:], attT[m][:, ts_], wo[m][:, cs],
                                         start=(m == 0), stop=(m == 3))
                    nc.scalar.copy(ot[:, cs], ps[:])
                nc.sync.dma_start(out_d[ts_, :], ot[:])

    nc.compile()
    names = dict(xt=xt_d.name, wqk=wqk_d.name, wv=wv_d.name, wo=wo_d.name,
                 ones=ones_d.name, out=out_d.name)
    return nc, names


def _prep_in_maps(x, ln_g, ln_b, w_qkv, w_out, b_out, names):
    x = np.asarray(x, dtype=np.float32)
    ln_g = np.asarray(ln_g, dtype=np.float64)
    ln_b = np.asarray(ln_b, dtype=np.float64)
    w_qkv = np.asarray(w_qkv, dtype=np.float64)
    w_out = np.asarray(w_out, dtype=np.float32)
    b_out = np.asarray(b_out, dtype=np.float32)

    wq = w_qkv[:, 0 * D:1 * D] * SCALE   # fold softmax scale into q
    wk = w_qkv[:, 1 * D:2 * D]
    wv_ = w_qkv[:, 2 * D:3 * D]

    in_maps = []
    for c in range(NCORES):
        b, g = c // 2, c % 2
        heads = range(g * HL, (g + 1) * HL)
        # wqk cols: [q_h0 q_h1 | ... | q_h6 q_h7 | k_h0 k_h1 | ...] pair tiles
        qcols = np.concatenate([wq[:, h * DH:(h + 1) * DH] for h in heads], axis=1)
        kcols = np.concatenate([wk[:, h * DH:(h + 1) * DH] for h in heads], axis=1)
        vcols = np.concatenate([wv_[:, h * DH:(h + 1) * DH] for h in heads], axis=1)
        wqk_c = np.concatenate([qcols, kcols], axis=1)          # [D, 1024]
        wqk_aug = np.concatenate([
            ln_g[:, None] * wqk_c,
            (ln_g @ wqk_c)[None, :],
            (ln_b @ wqk_c)[None, :]], axis=0)                   # [D+2, 1024]
        wv_aug = np.concatenate([
            ln_g[:, None] * vcols,
            (ln_g @ vcols)[None, :],
            (ln_b @ vcols)[None, :]], axis=0)                   # [D+2, 512]
        wo_c = w_out[g * IN_L:(g + 1) * IN_L, :]                # [512, D]
        in_maps.append({
            names["xt"]: np.ascontiguousarray(x[b].T).astype(BF16),
            names["wqk"]: wqk_aug.astype(np.float32).astype(BF16),
            names["wv"]: wv_aug.astype(np.float32).astype(BF16),
            names["wo"]: np.ascontiguousarray(wo_c).astype(BF16),
            names["ones"]: np.ones((1, N), dtype=BF16),
        })
    return in_maps


def _gather(res, names, b_out):
    out = np.empty((B, N, D), dtype=np.float32)
    for b in range(B):
        out[b] = res.results[2 * b][names["out"]] \
            + res.results[2 * b + 1][names["out"]] + np.asarray(b_out, np.float32)
    return out


def kernel(x, ln_g, ln_b, w_qkv, w_out, b_out):
    global _compiled
    from concourse.bass_utils import run_bass_kernel_spmd

    if _compiled is None:
        _compiled = _build()
    nc, names = _compiled
    in_maps = _prep_in_maps(x, ln_g, ln_b, w_qkv, w_out, b_out, names)
    res = run_bass_kernel_spmd(nc, in_maps, core_ids=list(range(NCORES)))
    return _gather(res, names, b_out)


def run_traced(x, ln_g, ln_b, w_qkv, w_out, b_out):
    """Test helper: run with NTFF tracing enabled, return BassKernelResults."""
    global _compiled
    from concourse.bass_utils import run_bass_kernel_spmd

    if _compiled is None:
        _compiled = _build()
    nc, names = _compiled
    in_maps = _prep_in_maps(x, ln_g, ln_b, w_qkv, w_out, b_out, names)
    return run_bass_kernel_spmd(nc, in_maps, core_ids=list(range(NCORES)),
                                trace=True)
